# revision 1
# baseline (speedup 1.0000x reference)
"""Brute-force KNN retrieval (B=512 queries, N=500000 candidates, D=128, top-K)
on 8 Trainium2 NeuronCores.

Strategy (sharding_hint): candidates sharded along N across the 8 cores,
queries replicated. Per core:
  - PE computes bf16 scores (fp32 PSUM) for its 62500-candidate shard.
  - ACT casts each 2048-wide PSUM chunk to fp16(score+128) and writes it
    into the HIGH int16 lanes of a persistent fp32 "packed" scan tile whose
    LOW int16 lanes hold a one-time iota (0..2047). For positive floats the
    fp32 bit pattern is monotone, so each packed fp32 orders by
    (fp16 score, then index).
  - DVE max8 reduces each packed chunk to its top-8 (values AND indices in
    one pass - no max_index / second scan needed).
Keep-top-8-per-2048-chunk is a safe reduction for top-100-of-500000
(P[>8 of a row's top-100 in one chunk] ~ 2e-8, plus tiny fp16-tie effects).
The host decodes survivors, rescores the top ~256 per row exactly in fp32,
and emits the exact global top-K (ties -> lower index, like lax.top_k).
"""

import sys

for _p in ("/opt/trn_rl_repo",):
    if _p not in sys.path:
        sys.path.insert(0, _p)

import numpy as np

B, N, D = 512, 500000, 128
N_CORES = 8
SHARD = N // N_CORES          # 62500 candidates per core
PCHUNK = 2048                 # PSUM tile width (4 banks) = max8 chunk
NCHUNK = -(-SHARD // PCHUNK)  # 31
PADN = PCHUNK * NCHUNK        # 63488 (padded shard width)
NSUB = PCHUNK // 512          # 4 matmuls per PSUM tile
MTILES = B // 128             # 4 query tiles
SURV = NCHUNK * 8             # 248 survivors per (row, core)
SCAN_BUFS = 4                 # persistent packed scan tiles (iota-carrying)
RESCORE = 256                 # host rescores this many per row exactly
BIAS = 128.0                  # score bias -> positive range for bit-ordering

_NC_CACHE = {}


def _build_nc():
    import concourse.bacc as bacc
    import concourse.tile as tile
    import concourse.mybir as mybir

    f32 = mybir.dt.float32
    f16 = mybir.dt.float16
    u16 = mybir.dt.uint16
    bf16 = mybir.dt.bfloat16

    nc = bacc.Bacc(
        "TRN2", target_bir_lowering=False, debug=False, num_devices=N_CORES
    )
    qT = nc.dram_tensor("qT", [D, B], bf16, kind="ExternalInput")
    cT = nc.dram_tensor("cT", [D, PADN], bf16, kind="ExternalInput")
    packed = nc.dram_tensor("packed", [B, SURV], f32, kind="ExternalOutput")

    with tile.TileContext(nc) as tc:
        with (
            tc.tile_pool(name="q", bufs=1) as qp,
            tc.tile_pool(name="c", bufs=4) as cp,
            tc.tile_pool(name="ps", bufs=2, space="PSUM") as pp,
            tc.tile_pool(name="scan", bufs=1) as sp,
            tc.tile_pool(name="acc", bufs=1) as op,
        ):
            qt = qp.tile([128, B], bf16)
            nc.sync.dma_start(qt[:], qT.ap())

            pacc = [
                op.tile([128, SURV], f32, name=f"pacc{m}", tag=f"p{m}")
                for m in range(MTILES)
            ]
            scan = [
                sp.tile([128, PCHUNK], f32, name=f"scan{j}", tag=f"s{j}")
                for j in range(SCAN_BUFS)
            ]
            # one-time iota into the LOW int16 lane of each packed fp32
            for j in range(SCAN_BUFS):
                lo = scan[j][:].bitcast(u16).rearrange(
                    "p (n two) -> p n two", two=2
                )[:, :, 0]
                nc.gpsimd.iota(lo, pattern=[[1, PCHUNK]], base=0,
                               channel_multiplier=0)

            for c in range(NCHUNK):
                ct = cp.tile([128, PCHUNK], bf16, name=f"ct{c}", tag="ct")
                nc.sync.dma_start(ct[:], cT.ap()[:, c * PCHUNK:(c + 1) * PCHUNK])
                for m in range(MTILES):
                    ps = pp.tile([128, PCHUNK], f32, name=f"ps{c}_{m}", tag="ps")
                    for s in range(NSUB):
                        nc.tensor.matmul(
                            ps[:, s * 512:(s + 1) * 512],
                            qt[:, m * 128:(m + 1) * 128],
                            ct[:, s * 512:(s + 1) * 512],
                            start=True,
                            stop=True,
                        )
                    sj = scan[(c * MTILES + m) % SCAN_BUFS]
                    hi = sj[:].bitcast(f16).rearrange(
                        "p (n two) -> p n two", two=2
                    )[:, :, 1]
                    nc.scalar.activation(
                        hi, ps[:], mybir.ActivationFunctionType.Copy,
                        bias=BIAS, scale=1.0,
                    )
                    nc.vector.max(pacc[m][:, c * 8:(c + 1) * 8], sj[:])

            for m in range(MTILES):
                nc.sync.dma_start(packed.ap()[m * 128:(m + 1) * 128, :], pacc[m][:])

    nc.compile()
    return nc


def _get_nc():
    if "nc" not in _NC_CACHE:
        _NC_CACHE["nc"] = _build_nc()
    return _NC_CACHE["nc"]


def _make_in_maps(queries, candidates):
    import ml_dtypes

    bf = ml_dtypes.bfloat16
    q = np.asarray(queries, dtype=np.float32)
    cand = np.asarray(candidates, dtype=np.float32)
    qTh = np.ascontiguousarray(q.T.astype(bf))  # [D, B] bf16
    in_maps = []
    for i in range(N_CORES):
        cTi = np.zeros((D, PADN), dtype=bf)
        cTi[:, :SHARD] = cand[i * SHARD:(i + 1) * SHARD].T.astype(bf)
        in_maps.append({"qT": qTh, "cT": cTi})
    return in_maps


def _run_device(in_maps, trace=False):
    from concourse import bass_utils

    nc = _get_nc()
    return bass_utils.run_bass_kernel_spmd(
        nc, in_maps, core_ids=list(range(N_CORES)), trace=trace
    )


def _merge(results, queries, candidates, identifiers, num_candidates):
    K = int(num_candidates)
    q = np.asarray(queries, dtype=np.float32)
    cand = np.asarray(candidates, dtype=np.float32)
    chunk_base = np.repeat(np.arange(NCHUNK, dtype=np.int64) * PCHUNK, 8)  # [SURV]
    all_u = []
    all_g = []
    for i in range(N_CORES):
        u = np.asarray(results[i]["packed"]).view(np.uint32)       # [B, SURV]
        local = chunk_base[None, :] + (u & 0xFFFF)                 # [B, SURV]
        valid = local < SHARD
        u = np.where(valid, u, 0)  # pads rank last
        g = i * SHARD + np.minimum(local, SHARD - 1)
        all_u.append(u)
        all_g.append(g)
    ucat = np.concatenate(all_u, axis=1)   # [B, 8*SURV] packed (monotone rank)
    gcat = np.concatenate(all_g, axis=1)
    # candidate set for exact rescoring: top RESCORE per row by packed rank
    nres = min(RESCORE, ucat.shape[1])
    part = np.argpartition(ucat, ucat.shape[1] - nres, axis=1)[:, -nres:]
    rows = np.arange(B)[:, None]
    gsel = gcat[rows, part]                                        # [B, nres]
    # exact fp32 rescore: s[b, j] = q[b] . cand[gsel[b, j]]
    csel = cand[gsel]                                              # [B, nres, D]
    vsel = np.einsum("bjd,bd->bj", csel, q, dtype=np.float32)
    # exact top-K, ties -> lower global index (matches lax.top_k)
    order = np.lexsort((gsel, -vsel), axis=-1)[:, :K]
    out_vals = np.take_along_axis(vsel, order, axis=1).astype(np.float32)
    out_gidx = np.take_along_axis(gsel, order, axis=1)
    ids = np.asarray(identifiers)
    out_ids = np.take(ids, out_gidx, axis=0)
    return out_vals, out_ids


def kernel(queries, candidates, identifiers, num_candidates):
    in_maps = _make_in_maps(queries, candidates)
    res = _run_device(in_maps, trace=False)
    return _merge(res.results, queries, candidates, identifiers, num_candidates)



# revision 3
# speedup vs baseline: 1.2622x; 1.2622x over previous
"""Brute-force KNN retrieval (B=512 queries, N=500000 candidates, D=128, top-K)
on 8 Trainium2 NeuronCores.

Strategy: candidates sharded along N across the 8 cores, queries replicated.
Per core, per (chunk c, query-tile m) PSUM tile of 2048 fp32 scores:
  - PE computes bf16 scores (fp32 PSUM).
  - The tile is reduced to 512 f16 "block maxima" (blocks of 4 candidates at
    positions {j, j+512, j+1024, j+1536}) using only ACT + DVE, split so both
    engines stay busy:
      * 3 of 4 tiles ("std"): ACT copies the hi half [1024:2048] to SBUF f16;
        DVE tensor_tensor-max folds the PSUM lo half against that copy
        (1 elem/cycle, the fold is free); one more f16 TT-max level (2x mode)
        gives the 512 block maxima.
      * 1 of 4 tiles ("full"): ACT copies all 2048; DVE does two f16 TT-max
        levels (both at 2x).
  - Block maxima are DMA'd out as f16 [B, 31*512] per core.
The host selects the top-R blocks per row (a block's max is >= every member's
score, so the <=100 blocks holding the true top-100 are always within the
top-R for R >= ~150; we use 384), rescores all <=4*R member candidates
exactly in fp32, and emits the exact global top-K (ties -> lower index,
matching lax.top_k).
"""

import sys

for _p in ("/opt/trn_rl_repo",):
    if _p not in sys.path:
        sys.path.insert(0, _p)

import numpy as np

B, N, D = 512, 500000, 128
K_MAX = 100
N_CORES = 8
SHARD = N // N_CORES          # 62500 candidates per core
PCHUNK = 2048                 # PSUM tile width (4 banks)
NCHUNK = -(-SHARD // PCHUNK)  # 31
PADN = PCHUNK * NCHUNK        # 63488 (padded shard width)
NSUB = PCHUNK // 512          # 4 matmuls per PSUM tile
MTILES = B // 128             # 4 query tiles
SPC = PCHUNK // 4             # 512 survivors (block maxima) per tile
SURV = NCHUNK * SPC           # 15872 survivors per (row, core)
RBLOCKS = 384                 # host rescores this many blocks per row

_NC_CACHE = {}


def _build_nc():
    import concourse.bacc as bacc
    import concourse.tile as tile
    import concourse.mybir as mybir

    f32 = mybir.dt.float32
    f16 = mybir.dt.float16
    bf16 = mybir.dt.bfloat16
    mx = mybir.AluOpType.max

    nc = bacc.Bacc(
        "TRN2", target_bir_lowering=False, debug=False, num_devices=N_CORES
    )
    qT = nc.dram_tensor("qT", [D, B], bf16, kind="ExternalInput")
    cT = nc.dram_tensor("cT", [D, PADN], bf16, kind="ExternalInput")
    surv = nc.dram_tensor("surv", [B, SURV], f16, kind="ExternalOutput")

    with tile.TileContext(nc) as tc:
        with (
            tc.tile_pool(name="q", bufs=1) as qp,
            tc.tile_pool(name="c", bufs=4) as cp,
            tc.tile_pool(name="ps", bufs=2, space="PSUM") as pp,
            tc.tile_pool(name="a", bufs=3) as ap_,
            tc.tile_pool(name="s1", bufs=3) as s1p,
            tc.tile_pool(name="s2", bufs=4) as s2p,
        ):
            qt = qp.tile([128, B], bf16)
            nc.sync.dma_start(qt[:], qT.ap())

            for c in range(NCHUNK):
                ct = cp.tile([128, PCHUNK], bf16, name=f"ct{c}", tag="ct")
                nc.sync.dma_start(ct[:], cT.ap()[:, c * PCHUNK:(c + 1) * PCHUNK])
                for m in range(MTILES):
                    ps = pp.tile([128, PCHUNK], f32, name=f"ps{c}_{m}", tag="ps")
                    for s in range(NSUB):
                        nc.tensor.matmul(
                            ps[:, s * 512:(s + 1) * 512],
                            qt[:, m * 128:(m + 1) * 128],
                            ct[:, s * 512:(s + 1) * 512],
                            start=True,
                            stop=True,
                        )
                    s1 = s1p.tile([128, 1024], f16, name=f"s1_{c}_{m}", tag="s1")
                    s2 = s2p.tile([128, SPC], f16, name=f"s2_{c}_{m}", tag="s2")
                    if m == 3:
                        # full path: ACT drains all 2048, DVE does 2 f16 levels
                        a = ap_.tile([128, 2048], f16, name=f"a{c}_{m}", tag="a")
                        nc.scalar.activation(
                            a[:], ps[:], mybir.ActivationFunctionType.Copy,
                            bias=0.0, scale=1.0,
                        )
                        nc.vector.tensor_tensor(
                            s1[:], a[:, 0:1024], a[:, 1024:2048], op=mx
                        )
                    else:
                        # std path: ACT drains hi half, DVE folds lo half
                        a = ap_.tile([128, 2048], f16, name=f"a{c}_{m}", tag="a")
                        nc.scalar.activation(
                            a[:, 0:1024], ps[:, 1024:2048],
                            mybir.ActivationFunctionType.Copy,
                            bias=0.0, scale=1.0,
                        )
                        nc.vector.tensor_tensor(
                            s1[:], ps[:, 0:1024], a[:, 0:1024], op=mx
                        )
                    nc.vector.tensor_tensor(
                        s2[:], s1[:, 0:512], s1[:, 512:1024], op=mx
                    )
                    nc.sync.dma_start(
                        surv.ap()[m * 128:(m + 1) * 128, c * SPC:(c + 1) * SPC],
                        s2[:],
                    )

    nc.compile()
    return nc


def _get_nc():
    if "nc" not in _NC_CACHE:
        _NC_CACHE["nc"] = _build_nc()
    return _NC_CACHE["nc"]


def _make_in_maps(queries, candidates):
    import ml_dtypes

    bf = ml_dtypes.bfloat16
    q = np.asarray(queries, dtype=np.float32)
    cand = np.asarray(candidates, dtype=np.float32)
    qTh = np.ascontiguousarray(q.T.astype(bf))  # [D, B] bf16
    in_maps = []
    for i in range(N_CORES):
        cTi = np.zeros((D, PADN), dtype=bf)
        cTi[:, :SHARD] = cand[i * SHARD:(i + 1) * SHARD].T.astype(bf)
        in_maps.append({"qT": qTh, "cT": cTi})
    return in_maps


def _run_device(in_maps, trace=False):
    from concourse import bass_utils

    nc = _get_nc()
    return bass_utils.run_bass_kernel_spmd(
        nc, in_maps, core_ids=list(range(N_CORES)), trace=trace
    )


def _merge(results, queries, candidates, identifiers, num_candidates):
    K = int(num_candidates)
    q = np.asarray(queries, dtype=np.float32)
    cand = np.asarray(candidates, dtype=np.float32)

    # survivor (core i, row b, k = c*SPC + j) -> block of 4 candidate ids:
    #   i*SHARD + c*PCHUNK + {j, j+512, j+1024, j+1536}
    vals = np.concatenate(
        [np.asarray(results[i]["surv"], dtype=np.float32) for i in range(N_CORES)],
        axis=1,
    )  # [B, 8*SURV]
    nblk = vals.shape[1]
    r = min(RBLOCKS, nblk)
    part = np.argpartition(vals, nblk - r, axis=1)[:, -r:]       # [B, r]
    core_of = part // SURV
    k_of = part % SURV
    c_of = k_of // SPC
    j_of = k_of % SPC
    # position within the core's padded shard; >= SHARD means zero padding
    pos = (c_of[:, :, None] * PCHUNK + j_of[:, :, None]
           + np.array([0, 512, 1024, 1536])[None, None, :])      # [B, r, 4]
    validity = pos < SHARD
    gids3 = core_of[:, :, None] * SHARD + np.minimum(pos, SHARD - 1)
    gids = gids3.reshape(B, -1)                                  # [B, 4r]
    valid = validity.reshape(B, -1)

    # exact fp32 rescore of all (<= 4r) member candidates per row
    out_vals = np.empty((B, K), dtype=np.float32)
    out_idx = np.empty((B, K), dtype=np.int64)
    bs = 128
    for b0 in range(0, B, bs):
        b1 = min(b0 + bs, B)
        g = gids[b0:b1]                                          # [bb, 4r]
        csel = cand[g]                                           # [bb, 4r, D]
        vsel = np.einsum("bjd,bd->bj", csel, q[b0:b1], dtype=np.float32)
        vsel = np.where(valid[b0:b1], vsel, -np.inf)
        # dedup duplicate ids within a row (blocks can't overlap, but pad
        # clamping can create dupes): keep first occurrence only
        order_g = np.argsort(g, axis=1, kind="stable")
        g_sorted = np.take_along_axis(g, order_g, axis=1)
        dup = np.zeros_like(g_sorted, dtype=bool)
        dup[:, 1:] = g_sorted[:, 1:] == g_sorted[:, :-1]
        dup_unsorted = np.zeros_like(dup)
        np.put_along_axis(dup_unsorted, order_g, dup, axis=1)
        vsel = np.where(dup_unsorted, -np.inf, vsel)
        order = np.lexsort((g, -vsel), axis=-1)[:, :K]
        out_vals[b0:b1] = np.take_along_axis(vsel, order, axis=1)
        out_idx[b0:b1] = np.take_along_axis(g, order, axis=1)

    ids = np.asarray(identifiers)
    out_ids = np.take(ids, out_idx, axis=0)
    return out_vals, out_ids


def kernel(queries, candidates, identifiers, num_candidates):
    in_maps = _make_in_maps(queries, candidates)
    res = _run_device(in_maps, trace=False)
    return _merge(res.results, queries, candidates, identifiers, num_candidates)
